# revision 34
# baseline (speedup 1.0000x reference)
"""Trainium2 Bass kernel for the BitwiseAutoencoder problem.

Pipeline (per core, data-parallel over batch: 8 of 64 batches per core):
  1. conv1d(1->256, k=256, stride=16, pad=256) as bf16 matmuls against a
     stride-replicated frame matrix R (one gather DMA per batch, resident).
     PSUM eviction (relu+bias, accum_out -> sum(h)) rotates over the
     Activation / DVE / Pool engines so the conv stays PE-paced; sum(h^2)
     per group via ACT Square-with-accum or DVE square+sum.
  2. [Sh, Sh2] all-gathered across the 8 cores; BN affine folded into the
     transposed-conv weights (a*W2, bf16) and a per-phase bias vector.
     fp32 filler matmuls keep the PE clock ramped through the collective.
  3. convT(256->1, k=256, stride=16) as bf16 matmuls; tap-half fold in PSUM
     via shifted rhs; (m, m+4) tap-group pairs folded on-chip with one
     64-partition DVE add (halves the DRAM bounce); remaining 4 groups are
     regrouped through a DRAM bounce + shifted gather and folded with a
     short add tree.  Output written phase-major, transposed on the host.

Self-contained: shapes/sharding hardcoded for x: [64, 1, 32768] f32, 8 cores.
"""

import numpy as np

import concourse.bass as bass
from concourse import bacc, mybir, tile
from concourse.bass_utils import run_bass_kernel_spmd

N_CORES = 8
B_FULL = 64
BPC = B_FULL // N_CORES  # 8 batches per core
T = 32768
K = 256
S = 16
BN_EPS = 1e-5

XP = T + 2 * K  # padded x length per batch (33280)
L = (T + 2 * K - K) // S + 1  # conv output length (2065)
RW = 2073  # R width: l in [0, 2064+8]
PW = XP // S  # 2080 phase columns

UW = 413  # conv matmul unit width (L = 5*413)
# conv eviction groups in units (per cc): small groups keep eviction latency
# under one group's matmul time so the conv stays PE-paced with 3 psum bufs
CONV_GROUPS = [1] + [2] * 19 + [1]  # 21 groups, 40 units per cc
CG_STARTS = [sum(CONV_GROUPS[:i]) for i in range(len(CONV_GROUPS))]
NGC = len(CONV_GROUPS)

# deconv output tiles over u' in [16, 2064); tail tiles shrink so the final
# bounce/gather/tree/out chain drains quickly after the last matmul
U_TILES = [(16, 342), (358, 342), (700, 342), (1042, 342), (1384, 342),
           (1726, 169), (1895, 169)]
W3A = 345  # allocated E width (wt + 3 max)
OFW2 = 349  # of2 free width (wt + 7 max)

DUMMY_N = 7  # fp32 filler matmuls covering the collective window

F32 = mybir.dt.float32
BF16 = mybir.dt.bfloat16
AF = mybir.ActivationFunctionType
ALU = mybir.AluOpType


def _flat_ap(tl, n0, dims):
    """Raw AP over an SBUF tile at flat free-offset n0 with given free dims."""
    full = tl[:]
    return bass.AP(tensor=full.tensor, offset=full.offset + n0,
                   ap=[[full.ap[0][0], 128]] + dims)


def _build():
    nc = bacc.Bacc("TRN2", target_bir_lowering=False, debug=False)

    # ---- external I/O ----
    xph_t = nc.dram_tensor("x_ph", [BPC, 16, PW], BF16, kind="ExternalInput")
    w1t_t = nc.dram_tensor("w1t", [128, 2, K], BF16, kind="ExternalInput")
    vecs_t = nc.dram_tensor("vecs", [128, 2, 3], F32, kind="ExternalInput")
    w2_t = nc.dram_tensor("w2", [128, 2, K], F32, kind="ExternalInput")
    w2fold_t = nc.dram_tensor("w2fold", [128, 2, 16], F32, kind="ExternalInput")
    cb16_t = nc.dram_tensor("cb16", [16], F32, kind="ExternalInput")
    y_t = nc.dram_tensor("y", [BPC, 16, 2048], F32, kind="ExternalOutput")

    with tile.TileContext(nc) as tc:
        with (
            tc.tile_pool(name="persist", bufs=1) as persist,
            tc.tile_pool(name="sqpool", bufs=2) as sqpool,
            tc.tile_pool(name="of2pool", bufs=2) as of2pool,
            tc.tile_pool(name="epool", bufs=2) as epool,
            tc.tile_pool(name="t4pool", bufs=2) as t4pool,
            tc.tile_pool(name="qpool", bufs=3) as qpool,
            tc.tile_pool(name="smalls", bufs=1) as smalls,
            tc.tile_pool(name="dram", bufs=1, space="DRAM") as dram,
        ):
            # ---- setup loads: weights + R frame matrices (all HWDGE) ----
            R = [persist.tile([128, RW], BF16, tag=f"R{b}", name=f"R{b}")
                 for b in range(BPC)]

            def load_r(b, eng):
                eng.dma_start(
                    out=R[b][:],
                    in_=bass.AP(tensor=xph_t, offset=b * XP,
                                ap=[[1, 8], [PW, 16], [1, RW]]),
                )

            load_r(0, nc.sync)
            w1t_sb = persist.tile([128, 2, K], BF16, tag="w1t")
            nc.scalar.dma_start(out=w1t_sb[:], in_=w1t_t[:, :, :])
            vecs_sb = persist.tile([128, 2, 3], F32, tag="vecs")
            nc.scalar.dma_start(out=vecs_sb[:], in_=vecs_t[:, :, :])
            for b in range(1, BPC):
                load_r(b, nc.sync)

            eps_sb = persist.tile([128, 1], F32, tag="eps")
            nc.vector.memset(eps_sb[:], BN_EPS)
            junkf = persist.tile([128, 512], F32, tag="junkf")
            nc.vector.memset(junkf[:], 0.125)
            junk = smalls.tile([128, 1], F32, tag="junk")
            # preload the Relu/Square activation table set while R loads run
            nc.scalar.activation(out=junk[:], in_=eps_sb[:], func=AF.Relu)

            # H: conv output (post-relu) bf16, flat layout (cc, b, l)
            H = persist.tile([128, 2, BPC, L], BF16, tag="H", name="H")
            sums1 = persist.tile([128, 2, NGC], F32, tag="s1", name="s1")
            sums2 = persist.tile([128, 2, NGC], F32, tag="s2", name="s2")
            tsjunk = persist.tile([128, 2 * UW], BF16, tag="tsj", name="tsj")

            # ================= phase 1: conv + stats =================
            # eviction engine rotation: Pool every other group, ACT/DVE
            # alternating in between (Pool is the slowest per element)
            ev_engines = [nc.gpsimd, nc.scalar, nc.gpsimd, nc.vector]
            with tc.tile_pool(name="psum_conv", bufs=4, space="PSUM") as pconv:
                for cc in range(2):
                    for gq in range(NGC):
                        gidx = cc * NGC + gq
                        nu_g = CONV_GROUPS[gq]
                        u0 = CG_STARTS[gq]
                        ps = pconv.tile([128, 2, 512], F32, tag="pc")
                        for i in range(nu_g):
                            w = u0 + i  # within-cc unit: 5*b + gi
                            b, gi = w // 5, w % 5
                            l0 = UW * gi
                            for h in range(2):
                                nc.tensor.matmul(
                                    ps[:, i, 0:UW],
                                    w1t_sb[:, h, 128 * cc:128 * (cc + 1)],
                                    R[b][:, l0 + 8 * h:l0 + 8 * h + UW],
                                    start=(h == 0), stop=(h == 1),
                                )
                        n0 = UW * u0 + 16520 * cc
                        out_ap = _flat_ap(H, n0, [[UW, nu_g], [1, UW]])
                        eng = ev_engines[gidx % 4]
                        if eng is nc.scalar:
                            nc.scalar.activation(
                                out=out_ap, in_=ps[:, 0:nu_g, 0:UW],
                                func=AF.Relu, bias=vecs_sb[:, cc, 0:1],
                                scale=1.0,
                                accum_out=sums1[:, cc, gq:gq + 1],
                            )
                        else:
                            # (psum + bias) max 0 -> bf16 H; accum -> sum(h)
                            eng.tensor_scalar(
                                out_ap, ps[:, 0:nu_g, 0:UW],
                                vecs_sb[:, cc, 0:1], 0.0,
                                ALU.add, ALU.max,
                                accum_out=sums1[:, cc, gq:gq + 1],
                            )
                        # sum(h^2) for this group
                        ncols = nu_g * UW
                        h_in = _flat_ap(H, n0, [[1, ncols]])
                        sq = sqpool.tile([128, 2 * UW], BF16, tag="sq",
                                         name=f"sq{gidx}")
                        if gidx % 14 in (0, 3, 6, 9, 12):
                            nc.scalar.activation(
                                out=sq[:, 0:ncols], in_=h_in, func=AF.Square,
                                accum_out=sums2[:, cc, gq:gq + 1],
                            )
                        else:
                            nc.vector.tensor_tensor(sq[:, 0:ncols], h_in,
                                                    h_in, ALU.mult)
                            nc.vector.tensor_scalar(
                                tsjunk[:, 0:ncols], sq[:, 0:ncols], 0.0, None,
                                ALU.add,
                                accum_out=sums2[:, cc, gq:gq + 1],
                            )

            # stats -> pk = [Sh0, Sh2_0, Sh1, Sh2_1]
            pk = smalls.tile([128, 4], F32, tag="pk")
            for cc in range(2):
                nc.vector.reduce_sum(pk[:, 2 * cc:2 * cc + 1],
                                     sums1[:, cc, :],
                                     axis=mybir.AxisListType.X)
                nc.vector.reduce_sum(pk[:, 2 * cc + 1:2 * cc + 2],
                                     sums2[:, cc, :],
                                     axis=mybir.AxisListType.X)
            bounce_in = dram.tile([128, 4], F32)
            bounce_out = dram.tile([N_CORES, 128, 4], F32)
            nc.sync.dma_start(out=bounce_in[:, :], in_=pk[:])
            # gate the PE filler matmuls on the stats being done: they read
            # junkf, whose first columns are rewritten here
            nc.vector.tensor_copy(junkf[:, 0:4], pk[:])

            # remaining weight loads + Sqrt/Copy table preload (ACT queue
            # drains phase-1 work first; ready well before the fold needs them)
            w2_sb = persist.tile([128, 2, K], F32, tag="w2")
            nc.scalar.dma_start(out=w2_sb[:], in_=w2_t[:, :, :])
            w2fold_sb = persist.tile([128, 2, 16], F32, tag="w2fold")
            nc.scalar.dma_start(out=w2fold_sb[:], in_=w2fold_t[:, :, :])
            cb_sb = persist.tile([16, 1], F32, tag="cb")
            nc.scalar.dma_start(out=cb_sb[:], in_=cb16_t[:])
            nc.scalar.activation(out=junk[:], in_=eps_sb[:], func=AF.Sqrt,
                                 bias=eps_sb[:, 0:1])

            # ================= phase 2: global BN =================
            nc.gpsimd.collective_compute(
                "AllGather",
                mybir.AluOpType.bypass,
                replica_groups=[list(range(N_CORES))],
                ins=[bounce_in.opt()],
                outs=[bounce_out.opt()],
            )
            # fp32 fillers keep the PE clock ramped through the collective;
            # their psum bank opens only now so the conv could use all 8
            pjunk_cm = tc.tile_pool(name="pjunk", bufs=1, space="PSUM")
            pjunk = pjunk_cm.__enter__()
            junkps = pjunk.tile([128, 512], F32, tag="jp")
            for _ in range(DUMMY_N):
                nc.tensor.matmul(junkps[:], junkf[:, 0:128], junkf[:],
                                 start=True, stop=True)
            pjunk_cm.__exit__(None, None, None)

            gall = smalls.tile([128, 4, N_CORES], F32, tag="gall")
            nc.sync.dma_start(
                out=gall[:],
                in_=bass.AP(tensor=bounce_out.tensor, offset=bounce_out.offset,
                            ap=[[4, 128], [1, 4], [512, N_CORES]]),
            )
            fold_prio = tc.high_priority()
            fold_prio.__enter__()
            gsum = smalls.tile([128, 4], F32, tag="gsum")
            nc.vector.reduce_sum(gsum[:], gall[:], axis=mybir.AxisListType.X)
            inv_n = 1.0 / (N_CORES * BPC * L)
            # fold BN scale into deconv weights -> bf16 (both cc vectorized;
            # cc = 0 first so the first deconv matmuls start earliest)
            mE = smalls.tile([128, 4], F32, tag="mE")
            nc.vector.tensor_scalar_mul(mE[:], gsum[:], inv_n)
            m2 = smalls.tile([128, 4], F32, tag="m2")
            nc.vector.tensor_mul(m2[:], mE[:], mE[:])
            mE_f = mE[:]
            m2_f = m2[:]
            mE_odd = bass.AP(tensor=mE_f.tensor, offset=mE_f.offset + 1,
                             ap=[[mE_f.ap[0][0], 128], [2, 2]])
            m2_even = bass.AP(tensor=m2_f.tensor, offset=m2_f.offset,
                              ap=[[m2_f.ap[0][0], 128], [2, 2]])
            mE_even = bass.AP(tensor=mE_f.tensor, offset=mE_f.offset,
                              ap=[[mE_f.ap[0][0], 128], [2, 2]])
            gv = smalls.tile([128, 2], F32, tag="gv")
            nc.vector.tensor_sub(gv[:], mE_odd, m2_even)
            sd = smalls.tile([128, 2], F32, tag="sd")
            nc.scalar.activation(out=sd[:], in_=gv[:], func=AF.Sqrt,
                                 bias=eps_sb[:, 0:1], scale=1.0)
            rinv = smalls.tile([128, 2], F32, tag="rinv")
            nc.vector.reciprocal(rinv[:], sd[:])
            vecs_f = vecs_sb[:]
            gamma = bass.AP(tensor=vecs_f.tensor, offset=vecs_f.offset + 1,
                            ap=[[vecs_f.ap[0][0], 128], [3, 2]])
            beta = bass.AP(tensor=vecs_f.tensor, offset=vecs_f.offset + 2,
                           ap=[[vecs_f.ap[0][0], 128], [3, 2]])
            aa = smalls.tile([128, 2], F32, tag="aa")
            nc.vector.tensor_mul(aa[:], rinv[:], gamma)
            w2a = persist.tile([128, 2, K], BF16, tag="w2a", name="w2a")
            for cc in range(2):
                nc.vector.tensor_scalar_mul(w2_sb[:, cc, :], w2_sb[:, cc, :],
                                            aa[:, cc:cc + 1])
                nc.vector.tensor_copy(w2a[:, cc, :], w2_sb[:, cc, :])
            admu = smalls.tile([128, 2], F32, tag="admu")
            nc.vector.tensor_mul(admu[:], aa[:], mE_even)
            dd = smalls.tile([128, 2], F32, tag="dd")
            nc.vector.tensor_sub(dd[:], beta, admu[:])
            fold_prio.__exit__(None, None, None)

            with (
                tc.tile_pool(name="psum_cp", bufs=1, space="PSUM") as psum_cp,
                tc.tile_pool(name="psum_dec", bufs=6, space="PSUM") as pdec,
            ):
                pcp = psum_cp.tile([16, 1], F32, tag="pcp")
                cp16 = smalls.tile([16, 1], F32, tag="cp16")
                cp_dram = dram.tile([16], F32)
                cpb = smalls.tile([128, 1], F32, tag="cpb")

                # ================= phase 3: deconv =================
                dbufs = [dram.tile([64, BPC, W3A], BF16, name=f"dbuf{i}")
                         for i in range(2)]
                dec_ev = [nc.vector, nc.scalar, nc.gpsimd]

                def tree(w0, wt, t4):
                    # fold the 4 tap groups: short bf16 tree, f32 root + bias
                    q0 = qpool.tile([128, 342], BF16, tag="q0",
                                    name=f"q0_{w0}")
                    q1 = qpool.tile([128, 342], BF16, tag="q1",
                                    name=f"q1_{w0}")
                    nc.vector.tensor_add(q0[:, 0:wt], t4[:, 0, 0:wt],
                                         t4[:, 1, 0:wt])
                    nc.gpsimd.tensor_add(q1[:, 0:wt], t4[:, 2, 0:wt],
                                         t4[:, 3, 0:wt])
                    ya = qpool.tile([128, 342], F32, tag="ya",
                                    name=f"ya_{w0}")
                    nc.vector.tensor_add(ya[:, 0:wt], q0[:, 0:wt],
                                         q1[:, 0:wt])
                    yb = qpool.tile([128, 342], F32, tag="yb",
                                    name=f"yb_{w0}")
                    nc.scalar.activation(out=yb[:, 0:wt], in_=ya[:, 0:wt],
                                         func=AF.Identity, bias=cpb[:, 0:1],
                                         scale=1.0)
                    return yb

                def y_out(w0, wt, yb):
                    nc.sync.dma_start(
                        out=bass.AP(tensor=y_t, offset=(w0 - 16),
                                    ap=[[2048, 16], [16 * 2048, 8], [1, wt]]),
                        in_=yb[:, 0:wt],
                    )

                pending_tree = None
                pending_y = None
                for ti, (w0, wt) in enumerate(U_TILES):
                    last = ti == len(U_TILES) - 1
                    w7 = wt + 7
                    w3 = wt + 3
                    dbuf = dbufs[ti % 2]
                    E = epool.tile([64, BPC, W3A], BF16, tag="E",
                                   name=f"E_{w0}")
                    for b in range(BPC):
                        if b % 2 == 0:
                            of2 = of2pool.tile([128, 2, OFW2], BF16, tag="OF2",
                                               name=f"of2_{w0}_{b}")
                        # tap-half fold in PSUM via shifted rhs
                        ps = pdec.tile([128, OFW2], F32, tag="pd")
                        nmm = 0
                        for kc in range(2):
                            for th, off in ((0, 7), (128, 15)):
                                nc.tensor.matmul(
                                    ps[:, 0:w7],
                                    w2a[:, kc, th:th + 128],
                                    H[:, kc, b, w0 - off:w0 - off + w7],
                                    start=(nmm == 0), stop=(nmm == 3),
                                )
                                nmm += 1
                        # per-phase bias matmul squeezed in right after the
                        # first deconv tile's first batch (needs dd)
                        if ti == 0 and b == 0:
                            nc.tensor.matmul(pcp[:], w2fold_sb[:, 0, :],
                                             dd[:, 0:1], start=True,
                                             stop=False)
                            nc.tensor.matmul(pcp[:], w2fold_sb[:, 1, :],
                                             dd[:, 1:2], start=False,
                                             stop=True)
                            nc.vector.tensor_add(cp16[:], pcp[:], cb_sb[:])
                            nc.sync.dma_start(out=cp_dram[:], in_=cp16[:])
                            # cpb[8p + b] = cp[p]
                            nc.sync.dma_start(
                                out=cpb[:],
                                in_=bass.AP(tensor=cp_dram.tensor,
                                            offset=cp_dram.offset,
                                            ap=[[1, 16], [0, 8], [0, 1]]),
                            )
                        # eviction rotates DVE / ACT / Pool
                        eng = dec_ev[b % 3]
                        dst = of2[:, b % 2, 0:w7]
                        if eng is nc.scalar:
                            nc.scalar.copy(dst, ps[:, 0:w7])
                        else:
                            eng.tensor_copy(dst, ps[:, 0:w7])
                        if b % 2 == 1:
                            # (m, m+4) pair fold: one 64-partition add
                            # E[16m+i, bp, v] = of2[16m+i, bp, v+4]
                            #                 + of2[16(m+4)+i, bp, v]
                            of2f = of2[:]
                            pstr = of2f.ap[0][0]
                            in0 = bass.AP(tensor=of2f.tensor,
                                          offset=of2f.offset + 4,
                                          ap=[[pstr, 64], [OFW2, 2], [1, w3]])
                            in1 = bass.AP(tensor=of2f.tensor,
                                          offset=of2f.offset + 64 * pstr,
                                          ap=[[pstr, 64], [OFW2, 2], [1, w3]])
                            nc.vector.tensor_tensor(
                                E[:, b - 1:b + 1, 0:w3], in0, in1, ALU.add)
                    if not last:
                        # bounce the folded taps to DRAM (HWDGE, one DMA)
                        nc.sync.dma_start(out=dbuf[:, :, 0:w3],
                                          in_=E[:, :, 0:w3])
                        # shifted gather: t4[8p+b, m, u] = E[16m+p, b, u+3-m]
                        dbf = dbuf[:]
                        t4 = t4pool.tile([128, 4, 342], BF16, tag="T4",
                                         name=f"t4_{w0}")
                        nc.sync.dma_start(
                            out=t4[:, :, 0:wt],
                            in_=bass.AP(
                                tensor=dbf.tensor, offset=dbf.offset + 3,
                                ap=[[BPC * W3A, 16], [W3A, 8],
                                    [16 * BPC * W3A - 1, 4], [1, wt]]),
                        )
                    # tile t's tree is emitted during tile t+1 and its y-out
                    # during tile t+2, so neither ever blocks the next tile's
                    # folds / bounce / gather at a queue head
                    if pending_y is not None:
                        y_out(*pending_y)
                        pending_y = None
                    if pending_tree is not None:
                        tw0, twt, tt4 = pending_tree
                        yb_prev = tree(tw0, twt, tt4)
                        if last:
                            y_out(tw0, twt, yb_prev)
                        else:
                            pending_y = (tw0, twt, yb_prev)
                        pending_tree = None
                    if not last:
                        pending_tree = (w0, wt, t4)
                        continue
                    # ---- final tile drains fully on-chip (no DRAM bounce):
                    # level 2: shuffle partners (m', m'+1) into place and add
                    #   E2[32a+p, b, w] = E[32a+p, b, w+1] + E[32a+16+p, b, w]
                    # level 3: (a, a+2) pairs via legal partition offsets
                    #   E3[p, b, x] = E2[p, b, x+2] + E2[32+p, b, x]
                    mask = [(i + 16) % 32 for i in range(32)]
                    esh = epool.tile([64, BPC, W3A], BF16, tag="esh",
                                     name="esh")
                    nc.vector.stream_shuffle(esh[:, :, 0:w3], E[:, :, 0:w3],
                                             mask)
                    e2 = epool.tile([64, BPC, W3A], BF16, tag="e2", name="e2")
                    ef = E[:]
                    epstr = ef.ap[0][0]
                    e_shift1 = bass.AP(tensor=ef.tensor, offset=ef.offset + 1,
                                       ap=[[epstr, 64], [W3A, BPC],
                                           [1, w3 - 1]])
                    nc.vector.tensor_tensor(e2[:, :, 0:w3 - 1], e_shift1,
                                            esh[:, :, 0:w3 - 1], ALU.add)
                    e2f = e2[:]
                    e2pstr = e2f.ap[0][0]
                    e2_hi = bass.AP(tensor=e2f.tensor,
                                    offset=e2f.offset + 32 * e2pstr,
                                    ap=[[e2pstr, 16], [W3A, BPC], [1, wt]])
                    e2_lo = bass.AP(tensor=e2f.tensor, offset=e2f.offset + 2,
                                    ap=[[e2pstr, 16], [W3A, BPC], [1, wt]])
                    e3 = smalls.tile([16, BPC, wt], F32, tag="e3", name="e3")
                    nc.vector.tensor_tensor(e3[:, :, 0:wt], e2_lo, e2_hi,
                                            ALU.add)
                    y4 = smalls.tile([16, BPC, wt], F32, tag="y4", name="y4")
                    nc.scalar.activation(out=y4[:, :, 0:wt],
                                         in_=e3[:, :, 0:wt],
                                         func=AF.Identity, bias=cp16[:, 0:1],
                                         scale=1.0)
                    nc.sync.dma_start(
                        out=bass.AP(tensor=y_t, offset=(w0 - 16),
                                    ap=[[2048, 16], [16 * 2048, 8], [1, wt]]),
                        in_=y4[:, :, 0:wt],
                    )
    nc.compile()
    return nc


_NC_CACHE = None


def _get_nc():
    global _NC_CACHE
    if _NC_CACHE is None:
        _NC_CACHE = _build()
    return _NC_CACHE


def _host_prep(inputs):
    import ml_dtypes

    conv_w = np.asarray(inputs["conv_w"], dtype=np.float32)
    conv_b = np.asarray(inputs["conv_b"], dtype=np.float32)
    conv_gate = np.asarray(inputs["conv_gate"], dtype=np.float32)
    conv_scale = np.asarray(inputs["conv_scale"], dtype=np.float32)
    bn_gamma = np.asarray(inputs["bn_gamma"], dtype=np.float32)
    bn_beta = np.asarray(inputs["bn_beta"], dtype=np.float32)
    ct_w = np.asarray(inputs["ct_w"], dtype=np.float32)
    ct_b = np.asarray(inputs["ct_b"], dtype=np.float32)
    ct_gate = np.asarray(inputs["ct_gate"], dtype=np.float32)
    ct_scale = np.asarray(inputs["ct_scale"], dtype=np.float32)

    W1 = conv_w[:, 0, :] * (conv_gate[:, 0, :] + 1.0) * 0.5  # [c, j]
    W1 = W1 * conv_scale[:, None]
    bias1 = conv_scale * conv_b
    # w1t[j0, h, c] = W1[c, j0 + 128h]
    w1t = np.ascontiguousarray(
        W1.T.reshape(2, 128, K).transpose(1, 0, 2)).astype(ml_dtypes.bfloat16)

    vecs = np.stack([bias1, bn_gamma, bn_beta], axis=1)  # [256, 3]
    vecs = np.ascontiguousarray(vecs.reshape(2, 128, 3).transpose(1, 0, 2))

    W2 = ct_w[:, 0, :] * (ct_gate[:, 0, :] + 1.0) * 0.5  # [k, j]
    W2 = W2 * float(ct_scale[0])
    w2 = np.ascontiguousarray(W2.reshape(2, 128, K).transpose(1, 0, 2))
    w2fold = W2.reshape(K, 16, 16).sum(axis=1)  # [k, p]
    w2fold = np.ascontiguousarray(w2fold.reshape(2, 128, 16).transpose(1, 0, 2))
    cb16 = np.full(16, float(ct_scale[0]) * float(ct_b[0]), dtype=np.float32)

    return {
        "w1t": w1t,
        "vecs": vecs.astype(np.float32),
        "w2": w2.astype(np.float32),
        "w2fold": w2fold.astype(np.float32),
        "cb16": cb16,
    }


def kernel(**inputs) -> np.ndarray:
    import ml_dtypes

    x = np.asarray(inputs["x"], dtype=np.float32)  # [64, 1, 32768]
    shared = _host_prep(inputs)
    nc = _get_nc()

    in_maps = []
    for c in range(N_CORES):
        shard = x[BPC * c:BPC * (c + 1), 0, :]  # [8, T]
        xpad = np.zeros((BPC, XP), dtype=np.float32)
        xpad[:, K:K + T] = shard
        # phase layout: x_ph[b, p, n] = x_pad[b, 16n + p]
        xph = np.ascontiguousarray(
            xpad.reshape(BPC, PW, 16).transpose(0, 2, 1)).astype(ml_dtypes.bfloat16)
        m = dict(shared)
        m["x_ph"] = xph
        in_maps.append(m)

    res = run_bass_kernel_spmd(nc, in_maps, core_ids=list(range(N_CORES)))
    outs = []
    for c in range(N_CORES):
        yph = res.results[c]["y"].reshape(BPC, 16, 2048)  # [b, p, u]
        outs.append(yph.transpose(0, 2, 1).reshape(BPC, 1, T))
    return np.concatenate(outs, axis=0).astype(np.float32)
